# revision 1
# baseline (speedup 1.0000x reference)
"""DevignLite GNN (3-layer GCN + dual pooling + MLP head) on 8 Trainium2 NeuronCores.

Strategy (v2)
-------------
- Nodes partitioned across 8 cores (12500 each, padded to 12544 = 98 blocks
  of 128).  Per layer the per-core table u = dinv * x is replicated with ONE
  AllGather into a 4-node-packed bf16 table [25088 rows x 256 cols], so SWDGE
  int16 gather indices (node//4) cover all 100352 node slots; the node%4 slot
  is resolved by a 128-col gather window (node%4//2) plus a 64-col lhsT slice
  (node%4%2).
- Aggregation is feature-major: per 128-edge tile, psum[64f, 128v] +=
  msg[128e, 64f].T @ S[128e, 128v] with S = one-hot(is_equal(iota, lid)) in
  bf16.  No forward transpose; dst-degree scaling is a column-wise multiply
  with a host-shipped packed dinv broadcast.
- Update: yT = W.T @ (dinv*z)T on PE (bf16), relu+bias on Act, transpose back
  + per-partition dinv scale + bf16 cast for the next table.
- Pooling: layer-2 activations accumulate feature-major in SBUF packed as
  [128, 6272] (two 6272-node lanes in the partition dim); segmented sum and
  max via single tensor_tensor_scan instructions (host masks reset state at
  graph starts; relu outputs make the 0-reset exact for max), per-segment
  extraction via host end-masks + static window sum-reduces + one-hot
  window->graph matmuls (lane/core-boundary splits resolved by add/max
  combining).  Partials AllGathered; every core computes the head identically.
"""

import sys

sys.path.insert(0, "/opt/trn_rl_repo")

import numpy as np
import ml_dtypes

P = 128
D = 64
BF = ml_dtypes.bfloat16


class Cfg:
    def __init__(self, N, E, V, G, n_cores=8, call_tiles=64):
        self.N, self.E, self.V, self.G = N, E, V, G
        self.NC = n_cores
        assert N % n_cores == 0
        self.NL = N // n_cores                      # real nodes per core
        self.NLP = -(-self.NL // P) * P             # padded
        assert self.NLP % (2 * P) == 0
        self.HL = self.NLP // 2                     # nodes per pooling lane
        self.B = self.NLP // P                      # dst blocks per core
        self.BH = self.B // 2                       # blocks per lane
        self.GRP = 4                                # dst blocks per gather call
        self.NGRP = -(-self.B // self.GRP)
        assert self.NLP % 4 == 0
        self.TROWS = n_cores * self.NLP // 4        # packed table rows
        assert self.TROWS < 32768
        self.VBS = min(V, 25000)                    # vocab block size
        self.NVB = -(-V // self.VBS)


# ----------------------------------------------------------------------------
# host-side preprocessing (structure only: bucketing, index streams, degrees)
# ----------------------------------------------------------------------------
def _preprocess(cfg, x_tokens, edge_index, batch):
    c = cfg
    N, NC, NL, NLP, B, HL = c.N, c.NC, c.NL, c.NLP, c.B, c.HL

    src = np.asarray(edge_index[0], dtype=np.int64)
    dst = np.asarray(edge_index[1], dtype=np.int64)
    loop = np.arange(N, dtype=np.int64)
    src = np.concatenate([src, loop])
    dst = np.concatenate([dst, loop])
    deg = np.bincount(dst, minlength=N).astype(np.float32)
    dinv = deg ** -0.5                              # deg >= 1 (self loops)

    # --- edge streams: cell = (slot, dst block), calls per (slot, group) ----
    oc = src // NL
    ol = src % NL
    row16 = (oc * (NLP // 4) + ol // 4).astype(np.int64)
    slot = (ol % 4).astype(np.int64)
    ecore = dst // NL
    ld = dst % NL
    blk = ld // P
    lid = (ld % P).astype(np.float32)

    cell = slot * B + blk                           # NCELL = 4 * B
    NCELL = 4 * B
    counts = np.zeros((NC, NCELL), dtype=np.int64)
    percore = []
    for ci in range(NC):
        m = ecore == ci
        cc = cell[m]
        order = np.argsort(cc, kind="stable")
        counts[ci] = np.bincount(cc, minlength=NCELL)
        percore.append((cc[order], row16[m][order], lid[m][order]))

    tiles_per_cell = -(-counts.max(axis=0) // P)    # shared across cores
    cell_tile_start = np.concatenate([[0], np.cumsum(tiles_per_cell)[:-1]])
    NT_TOT = int(tiles_per_cell.sum())

    # calls: one per (slot, group); emitted group-major to match consumption
    calls = []                                      # (t0, nt, half)
    for g in range(c.NGRP):
        for s in range(4):
            b0, b1 = g * c.GRP, min((g + 1) * c.GRP, B)
            t0 = int(cell_tile_start[s * B + b0])
            nt = int(tiles_per_cell[s * B + b0 : s * B + b1].sum())
            if nt > 0:
                calls.append((t0, nt, s // 2))
    MAXNT = max(nt for _, nt, _ in calls)

    # per-group consumption: groups -> [(blk, [(sub, t0, nt), ...]), ...]
    groups = []
    MAXCT = 1
    for g in range(c.NGRP):
        blist = []
        for b in range(g * c.GRP, min((g + 1) * c.GRP, B)):
            cells_b = []
            for s in range(4):
                ci0 = s * B + b
                st = int(cell_tile_start[ci0])
                nt = int(tiles_per_cell[ci0])
                if nt > 0:
                    cells_b.append((s % 2, st, nt))
                    MAXCT = max(MAXCT, nt)
            if cells_b:
                blist.append((b, cells_b))
        if blist:
            groups.append(blist)

    edge_idx_all = np.zeros((NC, NT_TOT * P), dtype=np.int16)
    edge_ids_all = np.full((NC, NT_TOT * P), -1.0, dtype=np.float32)
    for ci in range(NC):
        cc, ii, ll = percore[ci]
        within = np.arange(cc.size) - np.concatenate(
            [[0], np.cumsum(counts[ci])[:-1]]
        )[cc]
        pos = cell_tile_start[cc] * P + within
        edge_idx_all[ci, pos] = ii.astype(np.int16)
        edge_ids_all[ci, pos] = ll

    def wrap_idx(a):                                # [n] -> [128, n/16] int16
        n = a.size
        assert n % 16 == 0
        w = a.reshape(n // 16, 16).T
        return np.tile(w, (8, 1)).astype(np.int16)

    def tile_layout(a, fill, ncols):                # [n] -> [128, ncols]
        out = np.full((P, ncols), fill, dtype=np.float32)
        n = a.size
        t = np.arange(n) // P
        p = np.arange(n) % P
        out[p, t] = a
        return out

    edge_idx_w = np.stack([wrap_idx(edge_idx_all[ci]) for ci in range(NC)])
    edge_ids_t = np.stack(
        [tile_layout(edge_ids_all[ci], -1.0, NT_TOT) for ci in range(NC)]
    ).astype(BF)

    # --- embedding gather / scatter streams -------------------------------
    toks = np.asarray(x_tokens, dtype=np.int64).reshape(-1)
    vb = toks // c.VBS
    emb_cnt = np.zeros((NC, c.NVB), dtype=np.int64)
    for ci in range(NC):
        emb_cnt[ci] = np.bincount(vb[ci * NL : (ci + 1) * NL], minlength=c.NVB)
    EC = int(-(-emb_cnt.max() // P) * P)
    TRASH = NLP
    tok_idx = np.zeros((NC, c.NVB * EC), dtype=np.int16)
    tok_scat = np.full((NC, c.NVB * EC), TRASH, dtype=np.int16)
    dinv_perm = np.ones((NC, c.NVB * EC), dtype=np.float32)
    for ci in range(NC):
        tl = toks[ci * NL : (ci + 1) * NL]
        dl = dinv[ci * NL : (ci + 1) * NL]
        vbl = vb[ci * NL : (ci + 1) * NL]
        for b in range(c.NVB):
            rows = np.nonzero(vbl == b)[0]
            o = b * EC
            tok_idx[ci, o : o + rows.size] = (tl[rows] % c.VBS).astype(np.int16)
            tok_scat[ci, o : o + rows.size] = rows.astype(np.int16)
            dinv_perm[ci, o : o + rows.size] = dl[rows]

    tok_idx_w = np.stack([wrap_idx(tok_idx[ci]) for ci in range(NC)])
    tok_scat_w = np.stack([wrap_idx(tok_scat[ci]) for ci in range(NC)])
    ECC = EC // P
    dinv_perm_t = np.stack(
        [
            np.concatenate(
                [
                    tile_layout(dinv_perm[ci, b * EC : (b + 1) * EC], 1.0, ECC)
                    for b in range(c.NVB)
                ],
                axis=1,
            )
            for ci in range(NC)
        ]
    )

    # --- per-node dinv: lane-packed column broadcast + node-major ---------
    dinv_pad = np.ones((NC, NLP), dtype=np.float32)
    for ci in range(NC):
        dinv_pad[ci, :NL] = dinv[ci * NL : (ci + 1) * NL]

    dinv_bT = np.repeat(dinv_pad[:, None, :], D, axis=1).astype(BF)
    dinv_nm = np.stack([tile_layout(dinv_pad[ci], 1.0, B) for ci in range(NC)])

    # --- pooling: scan masks + window extraction (two lanes per core) -----
    batch = np.asarray(batch, dtype=np.int64)
    cnt = np.bincount(batch, minlength=c.G).astype(np.float32)
    rc = 1.0 / np.maximum(cnt, 1.0)
    rc_bcast = np.tile(rc[None, :], (D, 1)).astype(np.float32)  # [64, 256]

    # lane l of core ci covers nodes [l*HL, (l+1)*HL); the last real node of
    # lane 0 is col HL-1, of lane 1 is col NL-1-HL (both static).
    PL1 = c.NL - 1 - HL
    assert 0 <= PL1 < HL

    lane_bounds = [(0, HL, HL - 1), (HL, NL, PL1)]
    interior_all = [[], []]                          # per (core, lane) cols
    for ci in range(NC):
        bl = batch[ci * NL : (ci + 1) * NL]
        assert bl[HL - 1] != bl[NL - 1], "graph spans an entire lane"
        for li, (n0, n1, plc) in enumerate(lane_bounds):
            seg = bl[n0:n1]
            ends = np.nonzero(seg[1:] != seg[:-1])[0]   # interior ends < n1-n0-1
            interior_all[li].append(ends)

    for WW in (224, 112, 56, 32, 16):
        NW = HL // WW
        ok = True
        for li in range(2):
            for ci in range(NC):
                e = interior_all[li][ci]
                if e.size and np.bincount(e // WW, minlength=NW).max() > 1:
                    ok = False
                    break
            if not ok:
                break
        if ok:
            break
    assert ok, "window extraction infeasible"

    cont_m = np.zeros((NC, 2, HL), dtype=np.float32)
    Mwin = np.zeros((NC, NW + 2, 2 * c.G), dtype=np.float32)
    for ci in range(NC):
        bl = batch[ci * NL : (ci + 1) * NL]
        for li, (n0, n1, plc) in enumerate(lane_bounds):
            seg = bl[n0:n1]
            ln = n1 - n0
            same = np.zeros(ln, dtype=np.float32)
            same[1:] = (seg[1:] == seg[:-1]).astype(np.float32)
            cont_m[ci, li, :ln] = same
            ends = interior_all[li][ci]
            Mwin[ci, ends // WW, li * c.G + seg[ends]] = 1.0
            # wred col NW holds scan[:, HL-1] (lane0 last / lane1 garbage),
            # col NW+1 holds scan[:, PL1] (lane1 last / lane0 garbage).
            if li == 0:
                Mwin[ci, NW, li * c.G + seg[ln - 1]] = 1.0
            else:
                Mwin[ci, NW + 1, li * c.G + seg[ln - 1]] = 1.0

    def lane64(a):                                  # [NC, 2, HL] -> [NC,128,HL]
        return np.concatenate(
            [np.repeat(a[:, 0:1, :], D, axis=1),
             np.repeat(a[:, 1:2, :], D, axis=1)], axis=1
        )

    cont_pk = lane64(cont_m).astype(BF)

    meta = dict(
        NT_TOT=NT_TOT, EC=EC, calls=calls, cells=groups, MAXNT=MAXNT,
        MAXCT=MAXCT, WW=WW, NW=NW, PL1=PL1,
    )
    data = dict(
        edge_idx=edge_idx_w,
        edge_ids=edge_ids_t,
        tok_idx=tok_idx_w,
        tok_scat=tok_scat_w,
        dinv_perm=dinv_perm_t,
        dinv_bT=dinv_bT,
        dinv_nm=dinv_nm,
        cont_pk=cont_pk,
        Mwin=Mwin,
        rc_bcast=np.broadcast_to(rc_bcast, (NC, D, c.G)).copy(),
    )
    return meta, data


# ----------------------------------------------------------------------------
# the Bass/Tile program
# ----------------------------------------------------------------------------
def build_program(cfg, meta, repeat=1, skip=(), n_layers=3):
    import concourse.bacc as bacc
    import concourse.tile as tile
    from concourse import mybir
    from concourse.masks import make_identity

    c = cfg
    f32 = mybir.dt.float32
    bf16 = mybir.dt.bfloat16
    i16 = mybir.dt.int16
    AF = mybir.ActivationFunctionType
    OP = mybir.AluOpType
    AX = mybir.AxisListType
    NT_TOT, EC = meta["NT_TOT"], meta["EC"]
    CALLS, GROUPS, MAXNT = meta["calls"], meta["cells"], meta["MAXNT"]
    MAXCT = meta["MAXCT"]
    WW, NW, PL1 = meta["WW"], meta["NW"], meta["PL1"]
    ECC = EC // P
    NLP, B, G, HL, BH = c.NLP, c.B, c.G, c.HL, c.BH
    rg = [list(range(c.NC))]

    nc = bacc.Bacc("TRN2", target_bir_lowering=False, debug=False,
                   enable_asserts=False, num_devices=c.NC,
                   num_swdge_queues=4)

    import itertools
    from concourse.bass import AP as _AP
    pool_dma_q = itertools.count(8)

    def _bc_mid(ap, k):                  # [p, c] -> [p, k, c], stride-0 mid
        return _AP(ap.tensor, ap.offset, [ap.ap[0], [0, k], ap.ap[1]])

    def _bc_inner(ap, cct):              # [p, k] -> [p, k, cct], stride-0 inner
        return _AP(ap.tensor, ap.offset, [ap.ap[0], ap.ap[1], [0, cct]])

    emb = nc.dram_tensor("emb_table", [c.V, D], f32, kind="ExternalInput")
    edge_idx_d = nc.dram_tensor("edge_idx", [P, NT_TOT * 8], i16, kind="ExternalInput")
    edge_ids_d = nc.dram_tensor("edge_ids", [P, NT_TOT], bf16, kind="ExternalInput")
    tok_idx_d = nc.dram_tensor("tok_idx", [P, c.NVB * EC // 16], i16, kind="ExternalInput")
    tok_scat_d = nc.dram_tensor("tok_scat", [P, c.NVB * EC // 16], i16, kind="ExternalInput")
    dinv_perm_d = nc.dram_tensor("dinv_perm", [P, c.NVB * ECC], f32, kind="ExternalInput")
    dinv_bT_d = nc.dram_tensor("dinv_bT", [D, NLP], bf16, kind="ExternalInput")
    dinv_nm_d = nc.dram_tensor("dinv_nm", [P, B], f32, kind="ExternalInput")
    cont_pk_d = nc.dram_tensor("cont_pk", [P, HL], bf16, kind="ExternalInput")
    Mwin_d = nc.dram_tensor("Mwin", [NW + 2, 2 * G], f32, kind="ExternalInput")
    rc_d = nc.dram_tensor("rc_bcast", [D, G], f32, kind="ExternalInput")
    Ws = [nc.dram_tensor(f"W{i}", [D, D], f32, kind="ExternalInput") for i in range(3)]
    bs = [nc.dram_tensor(f"b{i}", [D], f32, kind="ExternalInput") for i in range(3)]
    Wc1_d = nc.dram_tensor("Wc1", [2 * D, D], f32, kind="ExternalInput")
    bc1_d = nc.dram_tensor("bc1", [D], f32, kind="ExternalInput")
    Wc2_d = nc.dram_tensor("Wc2", [D, 2], f32, kind="ExternalInput")
    bc2_d = nc.dram_tensor("bc2", [2], f32, kind="ExternalInput")
    logits_d = nc.dram_tensor("logits", [G, 2], f32, kind="ExternalOutput")

    u0f = nc.dram_tensor("u0f", [NLP + P, D], f32, kind="Internal")
    u_loc = [
        nc.dram_tensor(f"u{i}_loc", [NLP, D], bf16, kind="Internal")
        for i in range(3)
    ]
    u_full = [
        nc.dram_tensor(f"u{i}_full", [c.TROWS, 4 * D], bf16,
                       kind="Internal", addr_space="Shared")
        for i in range(3)
    ]
    pool_loc_d = nc.dram_tensor("pool_loc", [D, 2 * G], f32, kind="Internal")
    pool_all_d = nc.dram_tensor("pool_all", [c.NC * D, 2 * G], f32,
                                kind="Internal", addr_space="Shared")

    iota_bf_t = nc.inline_tensor(
        np.tile(np.arange(P, dtype=np.float32), (P, 1)).astype(BF), name="iota_bf"
    )
    iota_f_t = nc.inline_tensor(
        np.tile(np.arange(P, dtype=np.float32), (P, 1)), name="iota_f"
    )

    with tile.TileContext(nc) as tc:
        with (
            tc.tile_pool(name="persist", bufs=1) as pp,
            tc.tile_pool(name="msg", bufs=8) as msgp,
            tc.tile_pool(name="sel", bufs=4) as sp,
            tc.tile_pool(name="work", bufs=2) as wp,
            tc.tile_pool(name="ps", bufs=2, space="PSUM") as psp,
        ):
            # ---------- persistent SBUF state --------------------------------
            ids_sb = pp.tile([P, NT_TOT], bf16, tag="ids")
            nc.sync.dma_start(ids_sb[:], edge_ids_d[:])
            idx_sb = pp.tile([P, NT_TOT * 8], i16, tag="idx")
            nc.sync.dma_start(idx_sb[:], edge_idx_d[:])
            tok_idx_sb = pp.tile([P, c.NVB * EC // 16], i16, tag="tokidx")
            nc.sync.dma_start(tok_idx_sb[:], tok_idx_d[:])
            tok_scat_sb = pp.tile([P, c.NVB * EC // 16], i16, tag="tokscat")
            nc.sync.dma_start(tok_scat_sb[:], tok_scat_d[:])
            dinvp_sb = pp.tile([P, c.NVB * ECC], f32, tag="dinvp")
            nc.sync.dma_start(dinvp_sb[:], dinv_perm_d[:])
            dinv_bT_sb = pp.tile([D, NLP], bf16, tag="dinvbT")
            nc.sync.dma_start(dinv_bT_sb[:], dinv_bT_d[:])
            dinv_nm_sb = pp.tile([P, B], f32, tag="dinvnm")
            nc.sync.dma_start(dinv_nm_sb[:], dinv_nm_d[:])
            cm_sb = pp.tile([P, HL], bf16, tag="cm")
            nc.sync.dma_start(cm_sb[:], cont_pk_d[:])
            # end mask: interior end at c  <=>  col c+1 starts a new graph;
            # zero the final-end cols (extracted separately) and lane-1 pads.
            em_sb = pp.tile([P, HL], bf16, tag="em")
            nc.vector.tensor_scalar(
                em_sb[:, 0 : HL - 1], cm_sb[:, 1:HL], -1.0, 1.0,
                OP.mult, OP.add,
            )
            nc.vector.memset(em_sb[:, HL - 1 : HL], 0.0)
            nc.vector.memset(em_sb[D : 2 * D, PL1:HL], 0.0)
            Mwin_sb = pp.tile([NW + 2, 2 * G], f32, tag="Mwin")
            nc.sync.dma_start(Mwin_sb[:], Mwin_d[:])
            rc_sb = pp.tile([D, G], f32, tag="rc")
            nc.sync.dma_start(rc_sb[:], rc_d[:])
            iota_bf = pp.tile([P, P], bf16, tag="iotabf")
            nc.sync.dma_start(iota_bf[:], iota_bf_t[:])
            iota_f = pp.tile([P, P], f32, tag="iotaf")
            nc.sync.dma_start(iota_f[:], iota_f_t[:])
            ident_f = pp.tile([P, P], f32, tag="identf")
            make_identity(nc, ident_f[:])

            W_sb, b_sb = [], []
            for i in range(3):
                wf = wp.tile([D, D], f32, tag="wf")
                nc.sync.dma_start(wf[:], Ws[i][:])
                w = pp.tile([D, D], bf16, tag=f"W{i}")
                nc.vector.tensor_copy(w[:], wf[:])
                W_sb.append(w)
                b = pp.tile([D, 1], f32, tag=f"b{i}")
                nc.sync.dma_start(b[:], bs[i][:, None])
                b_sb.append(b)
            Wc1_sb = pp.tile([2 * D, D], f32, tag="Wc1")
            nc.sync.dma_start(Wc1_sb[:], Wc1_d[:])
            bc1_sb = pp.tile([D, 1], f32, tag="bc1")
            nc.sync.dma_start(bc1_sb[:], bc1_d[:, None])
            Wc2_sb = pp.tile([D, 2], f32, tag="Wc2")
            nc.sync.dma_start(Wc2_sb[:], Wc2_d[:])
            bc2_sb = pp.tile([2, 1], f32, tag="bc2")
            nc.sync.dma_start(bc2_sb[:], bc2_d[:, None])

            xT_pack = pp.tile([P, HL], bf16, tag="xT")

          # repeat wrapper (timing only; kernel() uses repeat=1)
          # fmt: off
            for _rep in range(repeat):
              nc.vector.memset(xT_pack[:], 0.0)

              # ---------- embedding: u0 = dinv * emb[tok] --------------------
              if "emb" not in skip:
                zt = wp.tile([P, 512], f32, tag="zt", bufs=1)
                nc.vector.memset(zt[:], 0.0)
                r = 0
                while r < NLP + P:
                    rows = min(1024, NLP + P - r)        # multiple of 128
                    nc.sync.dma_start(
                        u0f[r : r + rows, :].rearrange("(a b) c -> a (b c)", a=P),
                        zt[:, 0 : rows // P * D],
                    )
                    r += rows
                for b in range(c.NVB):
                    g = msgp.tile([P, ECC, D], f32, tag="msg")
                    nc.gpsimd.dma_gather(
                        g[:], emb[b * c.VBS : min((b + 1) * c.VBS, c.V), :],
                        tok_idx_sb[:, b * (EC // 16) : (b + 1) * (EC // 16)],
                        EC, EC, D, elem_step=D, single_packet=False,
                        queue_num=b,
                    )
                    sc = msgp.tile([P, ECC, D], f32, tag="msg")
                    nc.vector.tensor_tensor(
                        sc[:], g[:],
                        _bc_inner(dinvp_sb[:, b * ECC : (b + 1) * ECC], D),
                        OP.mult,
                    )
                    nc.gpsimd.dma_scatter_add(
                        u0f[:, :], sc[:],
                        tok_scat_sb[:, b * (EC // 16) : (b + 1) * (EC // 16)],
                        EC, EC, D, elem_step=D, single_packet=False,
                        queue_num=b,
                    )
                # cast u0 f32 -> bf16 table, in 7 chunks of 1792 rows
                CH = 896
                for k in range(NLP // CH):
                    ci_ = wp.tile([P, CH // P * D], f32, tag="castin", bufs=1)
                    nc.sync.dma_start(
                        ci_[:],
                        u0f[k * CH : (k + 1) * CH, :].rearrange(
                            "(a b) c -> a (b c)", a=P
                        ),
                    )
                    co_ = wp.tile([P, CH // P * D], bf16, tag="castout", bufs=1)
                    nc.vector.tensor_copy(co_[:], ci_[:])
                    nc.sync.dma_start(
                        u_loc[0][k * CH : (k + 1) * CH, :].rearrange(
                            "(a b) c -> a (b c)", a=P
                        ),
                        co_[:],
                    )
              if "coll" not in skip:
                  nc.gpsimd.collective_compute(
                      "AllGather", OP.bypass, replica_groups=rg,
                      ins=[u_loc[0][:, :]], outs=[u_full[0][:, :]],
                  )

              # ---------- GCN layers -------------------------------------------
              for layer in range(n_layers):
                  table = u_full[layer]
                  msg_ref = {}
                  for ci_call, (t0, nt, half) in enumerate(CALLS):
                      m = msgp.tile([P, MAXNT, 2 * D], bf16, tag="msg")
                      if "gather" in skip:
                          nc.sync.dma_start(
                              m[:, 0 : nt, :],
                              table[0 : nt * P, 0 : 2 * D].rearrange(
                                  "(t p) c -> p t c", p=P
                              ),
                          )
                      else:
                          nc.gpsimd.dma_gather(
                              m[:, 0:nt, :],
                              table[:, half * 2 * D : (half + 1) * 2 * D],
                              idx_sb[:, t0 * 8 : (t0 + nt) * 8],
                              nt * P, nt * P, 2 * D, elem_step=4 * D,
                              single_packet=False,
                              queue_num=next(pool_dma_q) % 4,
                          )
                      for j in range(nt):
                          msg_ref[t0 + j] = (m, j)
                  for blist in GROUPS:
                    for pi in range(0, len(blist), 2):
                      pair = blist[pi : pi + 2]
                      zbs = []
                      for (blk, cells_b) in pair:
                          zb = psp.tile([D, P], f32, tag="zb", bufs=4)
                          nt_ = sum(cnt for _, _, cnt in cells_b)
                          if "agg" in skip:
                              m, col = msg_ref[cells_b[0][1]]
                              nc.tensor.matmul(
                                  zb[:], m[:, col, 0:D], iota_bf[:],
                                  start=True, stop=True,
                              )
                          else:
                            j = 0
                            for (sub, ct0, cnt) in cells_b:
                              SK = sp.tile([P, MAXCT, P], bf16, tag="S")
                              nc.vector.tensor_tensor(
                                  SK[:, 0:cnt, :],
                                  _bc_mid(iota_bf[:], cnt),
                                  _bc_inner(ids_sb[:, ct0 : ct0 + cnt], P),
                                  OP.is_equal,
                              )
                              for k in range(cnt):
                                  m, col = msg_ref[ct0 + k]
                                  nc.tensor.matmul(
                                      zb[:],
                                      m[:, col, sub * D : (sub + 1) * D],
                                      SK[:, k, :],
                                      start=(j == 0), stop=(j == nt_ - 1),
                                  )
                                  j += 1
                          zbs.append(zb)
                      if "update" in skip:
                          continue
                      np_ = len(pair)
                      blk0 = pair[0][0]
                      zs = wp.tile([D, 2 * P], bf16, tag="zs")
                      for q, (blk, _) in enumerate(pair):
                          nc.vector.tensor_tensor(
                              zs[:, q * P : (q + 1) * P], zbs[q][:],
                              dinv_bT_sb[:, blk * P : (blk + 1) * P],
                              OP.mult,
                          )
                      yp = psp.tile([D, 2 * P], f32, tag="tp")
                      nc.tensor.matmul(yp[:], W_sb[layer][:],
                                       zs[:, 0 : np_ * P],
                                       start=True, stop=True)
                      oT = wp.tile([D, 2 * P], f32, tag="oT")
                      nc.scalar.activation(oT[:, 0 : np_ * P],
                                           yp[:, 0 : np_ * P], AF.Relu,
                                           bias=b_sb[layer][:])
                      if layer < 2:
                          tp = psp.tile([P, 2, D], f32, tag="tp")
                          for q in range(np_):
                              nc.tensor.transpose(
                                  tp[:, q, :], oT[:, q * P : (q + 1) * P],
                                  ident_f[0:D, 0:D],
                              )
                          un = wp.tile([P, 2, D], bf16, tag="un")
                          nc.vector.tensor_tensor(
                              un[:, 0:np_, :], tp[:, 0:np_, :],
                              _bc_inner(dinv_nm_sb[:, blk0 : blk0 + np_], D),
                              OP.mult,
                          )
                          nc.sync.dma_start(
                              u_loc[layer + 1][
                                  blk0 * P : (blk0 + np_) * P, :
                              ].rearrange("(t p) c -> p t c", p=P),
                              un[:, 0:np_, :],
                          )
                      else:
                          for q, (blk, _) in enumerate(pair):
                              lane = blk // BH
                              bcol = (blk % BH) * P
                              nc.vector.tensor_copy(
                                  xT_pack[lane * D : (lane + 1) * D,
                                          bcol : bcol + P],
                                  oT[:, q * P : (q + 1) * P],
                              )
                  if layer < 2 and "coll" not in skip:
                      nc.gpsimd.collective_compute(
                          "AllGather", OP.bypass, replica_groups=rg,
                          ins=[u_loc[layer + 1][:, :]],
                          outs=[u_full[layer + 1][:, :]],
                      )

              # ---------- pooling: segmented scan + window extraction ----------
              P_cat = wp.tile([D, 2 * G], f32, tag="pcat")
              if "pool" in skip or n_layers < 3:
                  nc.vector.memset(P_cat[:], 0.0)
              else:
               for kind, op1, comb_op, col0 in (
                   (0, OP.add, OP.add, 0), (1, OP.max, OP.max, G)
               ):
                   scan = wp.tile([P, HL], bf16, tag="scan", bufs=1)
                   nc.vector.tensor_tensor_scan(
                       scan[:], cm_sb[:], xT_pack[:], 0.0, OP.mult, op1
                   )
                   wred = wp.tile([P, NW + 2], f32, tag=f"wred{kind}")
                   nc.vector.tensor_copy(wred[:, NW : NW + 1], scan[:, HL - 1 : HL])
                   nc.vector.tensor_copy(
                       wred[:, NW + 1 : NW + 2], scan[:, PL1 : PL1 + 1]
                   )
                   nc.vector.tensor_tensor(scan[:], scan[:], em_sb[:], OP.mult)
                   nc.vector.tensor_reduce(
                       wred[:, 0:NW], scan[:].rearrange("p (w c) -> p w c", w=NW),
                       AX.X, OP.add,
                   )
                   wT_ps = psp.tile([NW + 2, P], f32, tag="ps")
                   nc.tensor.transpose(wT_ps[:], wred[:], ident_f[:])
                   wT = wp.tile([NW + 2, P], f32, tag=f"wT{kind}")
                   nc.vector.tensor_copy(wT[:], wT_ps[:])
                   pgw = psp.tile([D, G], f32, tag="ps")
                   nc.tensor.matmul(pgw[:], wT[:, 0:D], Mwin_sb[:, 0:G],
                                    start=True, stop=True)
                   pgl = psp.tile([D, G], f32, tag="ps")
                   nc.tensor.matmul(pgl[:], wT[:, D : 2 * D], Mwin_sb[:, G : 2 * G],
                                    start=True, stop=True)
                   nc.vector.tensor_copy(P_cat[:, col0 : col0 + G], pgw[:])
                   nc.vector.tensor_tensor(
                       P_cat[:, col0 : col0 + G], P_cat[:, col0 : col0 + G],
                       pgl[:], comb_op,
                   )
              nc.sync.dma_start(pool_loc_d[:, :], P_cat[:])
              if "coll" not in skip:
                  nc.gpsimd.collective_compute(
                      "AllGather", OP.bypass, replica_groups=rg,
                      ins=[pool_loc_d[:, :]], outs=[pool_all_d[:, :]],
                  )
              acc = wp.tile([D, 2 * G], f32, tag="acc")
              for rr in range(0, c.NC, 2):
                  pr = wp.tile([D, 2, 2 * G], f32, tag="pr", bufs=2)
                  nc.sync.dma_start(
                      pr[:],
                      pool_all_d[rr * D : (rr + 2) * D, :].rearrange(
                          "(r p) c -> p r c", p=D
                      ),
                  )
                  for q in range(2):
                      if rr + q == 0:
                          nc.vector.tensor_copy(acc[:], pr[:, 0, :])
                          continue
                      nc.vector.tensor_add(
                          acc[:, 0:G], acc[:, 0:G], pr[:, q, 0:G]
                      )
                      nc.vector.tensor_tensor(
                          acc[:, G : 2 * G], acc[:, G : 2 * G],
                          pr[:, q, G : 2 * G], OP.max,
                      )
              nc.vector.tensor_tensor(acc[:, 0:G], acc[:, 0:G], rc_sb[:], OP.mult)
              nc.vector.tensor_scalar(
                  acc[:, G : 2 * G], acc[:, G : 2 * G], 0.0, None, OP.max
              )
              hT = wp.tile([2 * D, G], f32, tag="hT")
              nc.vector.tensor_copy(hT[0:D, :], acc[:, 0:G])
              nc.vector.tensor_copy(hT[D : 2 * D, :], acc[:, G : 2 * G])
              # ---------- classifier head --------------------------------------
              h1 = psp.tile([D, G], f32, tag="ps")
              nc.tensor.matmul(h1[:], Wc1_sb[:], hT[:], start=True, stop=True)
              h1s = wp.tile([D, G], f32, tag="h1s")
              nc.scalar.activation(h1s[:], h1[:], AF.Relu, bias=bc1_sb[:])
              lg = psp.tile([2, G], f32, tag="ps")
              nc.tensor.matmul(lg[:], Wc2_sb[:], h1s[:], start=True, stop=True)
              lgs = wp.tile([2, G], f32, tag="lgs")
              nc.scalar.activation(lgs[:], lg[:], AF.Identity, bias=bc2_sb[:])
              for g in range(G // P):
                  lt = psp.tile([P, 2], f32, tag="ps")
                  nc.tensor.transpose(
                      lt[:], lgs[:, g * P : (g + 1) * P], ident_f[0:2, 0:2]
                  )
                  lts = wp.tile([P, 2], f32, tag="lts")
                  nc.vector.tensor_copy(lts[:], lt[:])
                  rows = min(G, (g + 1) * P) - g * P
                  nc.sync.dma_start(logits_d[g * P : g * P + rows, :], lts[0:rows, :])

          # fmt: on
    nc.compile()
    return nc


def make_in_maps(cfg, data, inputs):
    shared = {
        "emb_table": np.asarray(inputs["emb_table"], dtype=np.float32),
        "Wc1": np.asarray(inputs["Wc1"], dtype=np.float32),
        "bc1": np.asarray(inputs["bc1"], dtype=np.float32),
        "Wc2": np.asarray(inputs["Wc2"], dtype=np.float32),
        "bc2": np.asarray(inputs["bc2"], dtype=np.float32),
    }
    for i in range(3):
        shared[f"W{i}"] = np.asarray(inputs[f"W{i}"], dtype=np.float32)
        shared[f"b{i}"] = np.asarray(inputs[f"b{i}"], dtype=np.float32)
    in_maps = []
    for ci in range(cfg.NC):
        m = dict(shared)
        for k, v in data.items():
            m[k] = v[ci]
        in_maps.append(m)
    return in_maps


def kernel(**inputs):
    from concourse.bass_utils import run_bass_kernel_spmd

    x_tokens = np.asarray(inputs["x_tokens"])
    edge_index = np.asarray(inputs["edge_index"])
    batch = np.asarray(inputs["batch"])
    N = x_tokens.shape[0]
    E = edge_index.shape[1]
    V = np.asarray(inputs["emb_table"]).shape[0]
    G = 256
    cfg = Cfg(N, E, V, G)

    meta, data = _preprocess(cfg, x_tokens, edge_index, batch)
    nc = build_program(cfg, meta)
    in_maps = make_in_maps(cfg, data, inputs)
    res = run_bass_kernel_spmd(nc, in_maps, core_ids=list(range(cfg.NC)))
    return np.asarray(res.results[0]["logits"])

